# revision 35
# baseline (speedup 1.0000x reference)
"""Embedding lookup kernel for Trainium2 (8 NeuronCores, SPMD).

An embedding lookup IS a row gather: out[b, s, :] = weight[x[b, s], :].
Tokens are sharded 8 ways (1024 contiguous tokens per core); every core
keeps the full table in its DRAM. SHIPPED VARIANT (kernel() -> build_nc_v6
shared_idx/shared_g): 8 per-column vector-indirect DMAs.

Per core:
  1. One HWDGE DMA loads idx [128, 8] int32 into SBUF (idx[p, j] = token
     p*8 + j). Completion latency ~2.5-3.6us.
  2. EIGHT indirect_dma_start ops (InstDMACopy + DynamicAccessPatternInfo
     on the qPoolDynamic SWDGE queue): op j gathers w[idx[p, j]] -> 512B
     row into g[p, j*128:(j+1)*128] for all 128 partitions. The HW
     indirect1d expansion consumes exactly ONE index per DEST PARTITION
     per op (one desc per index, elem = the partition's contiguous span),
     so 1024 rows need 8 ops; each op occupies the Pool dispatch path
     ~1.41us SERIALLY (SEQ<->Q7 command handshake; the queue attr on
     InstDMACopy does NOT route across SWDGE queues - only the custom
     ucode instructions carry queue_num).
  3. Writebacks: cols 0-5 on Sync HWDGE as soon as their gathers land,
     cols 6-7 merged on the Activation HWDGE (parallel tail).

One-shot cost on HW ~23.4-24.3us: NEFF preamble ~5.7 (engine barriers +
TPB base loads + input-ready event, framework-emitted, not removable from
bass) + idx ~2.6-3.6 + 8x1.41 dispatch + drain/wb ~2.7 + epilogue ~1.2.

Measured dead ends (this session):
- ONE batched indirect op ([P, TPP] idx, flat 2D dest) runs 15.2us total
  but gathers w[idx[p,0]+j] (contiguous run per partition), not per-block
  indices - wrong for arbitrary tokens.
- DRAM->DRAM indirect gather (1024-entry dest, no writeback): crashes the
  runtime (the "Keyhan" bass assert is real).
- dma_gather (ucode): correctness-equal but its mlp LOAD_LIB costs 8.8us
  one-shot even hoisted first (v8 ~24.9); loop-amortized numbers from the
  v4 line do not transfer to the one-shot graded metric.
- ap_gather (vocab-sharded SBUF-resident table): ucode costs ~27ns/token
  at d=1 (41us for 1536 tokens); also any >=2MB DMA concurrent with a
  gpsimd library load starves the Q7 loader (2.3us -> 43us).

Constraints kept from the earlier session: DMA-completion semaphores are
per-queue; pow2 dma_gather chunk sizes; idx layouts are pre-wrapped on
the host. loop_m > 1 builds the timing-harness variant.
"""

import contextlib

import numpy as np

import concourse.bass as bass
from concourse import bacc, library_config, mybir
from concourse.bass_utils import run_bass_kernel_spmd

N_CORES = 8
B, S = 2, 4096
VOCAB, DIM = 32000, 128
P = 128
TOKENS = B * S                      # 8192
TPC = TOKENS // N_CORES             # 1024 tokens per core
TPP = TPC // P                      # 8 tokens per partition
IDX_COLS = TPC // 16                # 64 int16 idxs per partition row


def build_nc(loop_m: int = 1):
    # Skip the Bass-constructor entry barrier (gates the first DMA behind
    # all engines' init); restore the method right after construction.
    orig_barrier = bass.Bass.all_engine_barrier
    bass.Bass.all_engine_barrier = lambda self, *a, **k: None
    try:
        nc = bacc.Bacc(
            None, target_bir_lowering=False, dynamic_dma_scratch_size=32768
        )
    finally:
        bass.Bass.all_engine_barrier = orig_barrier

    x = nc.dram_tensor("x", [P, IDX_COLS], mybir.dt.int16, kind="ExternalInput")
    w = nc.dram_tensor("weight", [VOCAB, DIM], mybir.dt.float32, kind="ExternalInput")
    out = nc.dram_tensor("out", [P, TPP, DIM], mybir.dt.float32, kind="ExternalOutput")

    with contextlib.ExitStack() as ctx:
        idx_tile = ctx.enter_context(
            nc.sbuf_tensor("idx_tile", [P, IDX_COLS], mybir.dt.int16)
        )
        g = ctx.enter_context(nc.sbuf_tensor("g", [P, TPP, DIM], mybir.dt.float32))
        dummy_idx = ctx.enter_context(
            nc.sbuf_tensor("dummy_idx", [P, 8], mybir.dt.int16)
        )
        scratch = ctx.enter_context(
            nc.sbuf_tensor("scratch", [P, 1, DIM], mybir.dt.float32)
        )
        s_idx = ctx.enter_context(nc.semaphore("s_idx"))
        s_warm = ctx.enter_context(nc.semaphore("s_warm"))
        s_ms = ctx.enter_context(nc.semaphore("s_ms"))
        s_g = ctx.enter_context(nc.semaphore("s_g"))
        s_out = ctx.enter_context(nc.semaphore("s_out"))

        # Hoist the num_idxs register materialization off the critical path
        # (otherwise the mov lands after the s_idx wait).
        n_reg = nc.gpsimd.to_reg(TPC)

        # Warmup gather (128 zero indices), hidden inside the idx-DMA latency
        # window; also pulls the gpsimd library load off the critical path.
        nc.gpsimd.memset(dummy_idx[:], 0).then_inc(s_ms, 1)
        nc.gpsimd.wait_ge(s_ms, 1)
        nc.gpsimd.dma_gather(
            scratch[:], w[:], dummy_idx[:], P, P, DIM
        ).then_inc(s_warm, 16)

        # loop_m > 1 is the timing-harness mode: repeat the body with full
        # cross-iteration serialization (iter k+1's idx load waits for iter
        # k's writeback) so wall-time deltas measure per-iteration latency.
        for k in range(loop_m):
            if k > 0:
                nc.sync.wait_ge(s_out, 16 * k)
            nc.sync.dma_start(idx_tile[:], x[:]).then_inc(s_idx, 16)
            nc.gpsimd.wait_ge(s_idx, 16 * (k + 1))
            nc.gpsimd.dma_gather(
                g[:], w[:], idx_tile[:], TPC, n_reg, DIM
            ).then_inc(s_g, 16)
            nc.sync.wait_ge(s_g, 16 * (k + 1))
            nc.sync.dma_start(out[:], g[:]).then_inc(s_out, 16)
    nc.compile()
    return nc


def build_nc_v3(loop_m: int = 1):
    """v3: writeback via a prepared dma_scatter_add with iota indices.

    The scatter's descriptors (SBUF g -> DRAM out rows 0..1023) are generated
    on the Pool engine while the gather's data is still draining, then fired
    with trigger_dma as soon as the gather's completion semaphore arrives —
    removing the HWDGE dispatch from the tail. out rows are pre-zeroed by the
    runtime, so += is =. Output lands in natural token order [1024, 128].
    """
    orig_barrier = bass.Bass.all_engine_barrier
    bass.Bass.all_engine_barrier = lambda self, *a, **k: None
    try:
        nc = bacc.Bacc(
            None, target_bir_lowering=False, dynamic_dma_scratch_size=32768
        )
    finally:
        bass.Bass.all_engine_barrier = orig_barrier

    x = nc.dram_tensor("x", [P, IDX_COLS], mybir.dt.int16, kind="ExternalInput")
    wbx = nc.dram_tensor("wb_idx", [P, IDX_COLS], mybir.dt.int16, kind="ExternalInput")
    w = nc.dram_tensor("weight", [VOCAB, DIM], mybir.dt.float32, kind="ExternalInput")
    out = nc.dram_tensor("out", [TPC, DIM], mybir.dt.float32, kind="ExternalOutput")

    with contextlib.ExitStack() as ctx:
        idx_tile = ctx.enter_context(
            nc.sbuf_tensor("idx_tile", [P, IDX_COLS], mybir.dt.int16)
        )
        wbx_tile = ctx.enter_context(
            nc.sbuf_tensor("wbx_tile", [P, IDX_COLS], mybir.dt.int16)
        )
        g = ctx.enter_context(nc.sbuf_tensor("g", [P, TPP, DIM], mybir.dt.float32))
        dummy_idx = ctx.enter_context(
            nc.sbuf_tensor("dummy_idx", [P, 8], mybir.dt.int16)
        )
        scratch = ctx.enter_context(
            nc.sbuf_tensor("scratch", [P, 1, DIM], mybir.dt.float32)
        )
        s_idx = ctx.enter_context(nc.semaphore("s_idx"))
        s_wbx = ctx.enter_context(nc.semaphore("s_wbx"))
        s_warm = ctx.enter_context(nc.semaphore("s_warm"))
        s_ms = ctx.enter_context(nc.semaphore("s_ms"))
        s_g = ctx.enter_context(nc.semaphore("s_g"))
        s_wb = ctx.enter_context(nc.semaphore("s_wb"))
        s_prep = ctx.enter_context(nc.semaphore("s_prep"))

        nc.gpsimd.memset(dummy_idx[:], 0).then_inc(s_ms, 1)
        nc.gpsimd.wait_ge(s_ms, 1)
        nc.gpsimd.dma_gather(
            scratch[:], w[:], dummy_idx[:], P, P, DIM
        ).then_inc(s_warm, 16)

        nc.sync.dma_start(idx_tile[:], x[:]).then_inc(s_idx, 16)
        nc.sync.dma_start(wbx_tile[:], wbx[:]).then_inc(s_wbx, 16)

        for k in range(loop_m):
            if k > 0:
                nc.sync.wait_ge(s_wb, 16 * k)
                nc.sync.dma_start(idx_tile[:], x[:]).then_inc(s_idx, 16)
            nc.gpsimd.wait_ge(s_idx, 16 * (k + 1))
            nc.gpsimd.dma_gather(g[:], w[:], idx_tile[:], TPC, TPC, DIM).then_inc(
                s_g, 16
            )
            if k == 0:
                nc.gpsimd.wait_ge(s_wbx, 16)
            nc.gpsimd.dma_scatter_add(
                out[:], g[:], wbx_tile[:], TPC, TPC, DIM,
                prepare_only=True, sem=s_wb,
            ).then_inc(s_prep, 1)
            nc.gpsimd.wait_ge(s_prep, k + 1)
            nc.gpsimd.wait_ge(s_g, 16 * (k + 1))
            nc.gpsimd.trigger_dma(count=1)
        nc.gpsimd.wait_ge(s_wb, 16 * loop_m)
    nc.compile()
    return nc


def build_nc_v4(loop_m: int = 1, sizes=(512, 512), wb_engines=("sync",),
                n_queues: int = 1, warm_queues: int | None = None,
                warm_in_loop: bool = False, single_packet: bool = True,
                wb_groups=None):
    """v4: gather + writeback split into pipelined chunks of `sizes` tokens
    (each a multiple of 128). Chunk c's HWDGE writeback overlaps chunk c+1's
    gather desc-gen/drain, at the price of an extra ~1us SWDGE fixed overhead
    per extra chunk. wb_engines: round-robin engines for the writebacks
    ("sync" = SP, "act" = Activation). n_queues > 1 round-robins the gathers
    over that many SWDGE queues."""
    assert sum(sizes) == TPC and all(s % 128 == 0 for s in sizes)
    orig_barrier = bass.Bass.all_engine_barrier
    bass.Bass.all_engine_barrier = lambda self, *a, **k: None
    try:
        nc = bacc.Bacc(
            None, target_bir_lowering=False, dynamic_dma_scratch_size=32768,
            num_swdge_queues=n_queues, use_seq_codegen=seq_codegen,
        )
    finally:
        bass.Bass.all_engine_barrier = orig_barrier

    x = nc.dram_tensor("x", [P, IDX_COLS], mybir.dt.int16, kind="ExternalInput")
    w = nc.dram_tensor("weight", [VOCAB, DIM], mybir.dt.float32, kind="ExternalInput")
    out = nc.dram_tensor("out", [P, TPP, DIM], mybir.dt.float32, kind="ExternalOutput")

    chunks = len(sizes)
    bounds = [0]
    for s in sizes:
        bounds.append(bounds[-1] + s)

    with contextlib.ExitStack() as ctx:
        idx_tile = ctx.enter_context(
            nc.sbuf_tensor("idx_tile", [P, IDX_COLS], mybir.dt.int16)
        )
        g = ctx.enter_context(nc.sbuf_tensor("g", [P, TPP, DIM], mybir.dt.float32))
        dummy_idx = ctx.enter_context(
            nc.sbuf_tensor("dummy_idx", [P, 8], mybir.dt.int16)
        )
        scratch = ctx.enter_context(
            nc.sbuf_tensor("scratch", [P, max(n_queues, 1), DIM], mybir.dt.float32)
        )
        s_idx = ctx.enter_context(nc.semaphore("s_idx"))
        s_warms = [
            ctx.enter_context(nc.semaphore(f"s_warm{q}"))
            for q in range(max(warm_queues if warm_queues is not None else n_queues, 1))
        ]
        s_ms = ctx.enter_context(nc.semaphore("s_ms"))
        s_gs = [ctx.enter_context(nc.semaphore(f"s_g{c}")) for c in range(chunks)]
        s_out = ctx.enter_context(nc.semaphore("s_out"))

        n_regs = {}
        for s in dict.fromkeys(sizes):
            n_regs[s] = nc.gpsimd.to_reg(s)

        if warm_queues is None:
            # One warmup only: the q0 warmup absorbs the library load + ucode
            # cold cost. Extra per-queue warmups measured ~1.4us each and run
            # serially on the Pool engine, overrunning the ~2.3us idx-DMA
            # window — a guaranteed delay for an unproven per-queue saving.
            warm_queues = 1

        def emit_warmups():
            # One dummy gather per SWDGE queue: warms the ucode path, the
            # per-queue doorbell/ring state, and (queue 0, first) pulls the
            # library load off the critical path. Queue 0 gets the full
            # 128-idx warmup; the rest use 16 idxs (fixed cost dominates).
            for q in range(warm_queues):
                if q == 0:
                    nc.gpsimd.dma_gather(
                        scratch[:, 0:1, :], w[:], dummy_idx[:], P, P, DIM,
                        queue_num=0,
                    ).then_inc(s_warms[0], 16)
                else:
                    nc.gpsimd.dma_gather(
                        scratch[:, q : q + 1, :], w[:], dummy_idx[:, :1], 16, 16,
                        DIM, queue_num=q,
                    ).then_inc(s_warms[q], 16)

        nc.gpsimd.memset(dummy_idx[:], 0).then_inc(s_ms, 1)
        nc.gpsimd.wait_ge(s_ms, 1)
        emit_warmups()

        nc.sync.dma_start(idx_tile[:], x[:]).then_inc(s_idx, 16)

        engines = {"sync": nc.sync, "act": nc.scalar}

        n_wbs = len(wb_groups) if wb_groups else chunks
        for k in range(loop_m):
            if k > 0:
                nc.sync.wait_ge(s_out, 16 * n_wbs * k)
                nc.sync.dma_start(idx_tile[:], x[:]).then_inc(s_idx, 16)
            if warm_in_loop and k > 0:
                emit_warmups()
            nc.gpsimd.wait_ge(s_idx, 16 * (k + 1))
            for c in range(chunks):
                j0, j1 = bounds[c] // P, bounds[c + 1] // P
                nc.gpsimd.dma_gather(
                    g[:, j0:j1, :],
                    w[:],
                    idx_tile[:, bounds[c] // 16 : bounds[c + 1] // 16],
                    sizes[c],
                    n_regs[sizes[c]],
                    DIM,
                    queue_num=c % n_queues,
                    single_packet=single_packet,
                ).then_inc(s_gs[c], 16)
            groups = wb_groups or [(c,) for c in range(chunks)]
            for gi, grp in enumerate(groups):
                j0 = bounds[grp[0]] // P
                j1 = bounds[grp[-1] + 1] // P
                eng = engines[wb_engines[gi % len(wb_engines)]]
                for c in grp:
                    eng.wait_ge(s_gs[c], 16 * (k + 1))
                eng.dma_start(
                    out[:, j0:j1, :], g[:, j0:j1, :]
                ).then_inc(s_out, 16)
    nc.compile()
    return nc


def build_nc_v4b(loop_m: int = 1):
    return build_nc_v4(loop_m, sizes=(640, 384))


def build_nc_v4c(loop_m: int = 1):
    return build_nc_v4(loop_m, sizes=(512, 256, 256))


def build_nc_v4d(loop_m: int = 1):
    return build_nc_v4(loop_m, sizes=(256, 256, 256, 256))


def build_nc_v4c2(loop_m: int = 1):
    return build_nc_v4(loop_m, sizes=(512, 256, 256), wb_engines=("sync", "act"))


def build_nc_v4asc(loop_m: int = 1):
    return build_nc_v4(loop_m, sizes=(256, 256, 512))


def build_nc_v4ascq(loop_m: int = 1):
    return build_nc_v4(loop_m, sizes=(256, 256, 512), n_queues=3)


def build_nc_v4ascq_w(loop_m: int = 1):
    # probe: per-queue warmups re-run inside every loop iteration
    return build_nc_v4(loop_m, sizes=(256, 256, 512), n_queues=3,
                       warm_in_loop=True)


def build_nc_v4ascq2(loop_m: int = 1):
    return build_nc_v4(loop_m, sizes=(256, 256, 512), n_queues=2)


def build_nc_v4eq(loop_m: int = 1):
    return build_nc_v4(loop_m, sizes=(128, 128, 256, 512), n_queues=4)


def build_nc_v4gq(loop_m: int = 1):
    return build_nc_v4(loop_m, sizes=(128, 128, 128, 128, 512), n_queues=4)


def _probe_builder(loop_m: int, *, no_idx: bool = False, no_wb: bool = False,
                   sizes=(128, 128, 256, 512), n_queues: int = 4):
    """Timing probes: v4eq with the per-iteration idx DMA and/or the
    writebacks removed, to decompose per-iteration time on HW."""
    orig_barrier = bass.Bass.all_engine_barrier
    bass.Bass.all_engine_barrier = lambda self, *a, **k: None
    try:
        nc = bacc.Bacc(
            None, target_bir_lowering=False, dynamic_dma_scratch_size=32768,
            num_swdge_queues=n_queues, use_seq_codegen=seq_codegen,
        )
    finally:
        bass.Bass.all_engine_barrier = orig_barrier

    x = nc.dram_tensor("x", [P, IDX_COLS], mybir.dt.int16, kind="ExternalInput")
    w = nc.dram_tensor("weight", [VOCAB, DIM], mybir.dt.float32, kind="ExternalInput")
    out = nc.dram_tensor("out", [P, TPP, DIM], mybir.dt.float32, kind="ExternalOutput")

    chunks = len(sizes)
    bounds = [0]
    for s in sizes:
        bounds.append(bounds[-1] + s)

    with contextlib.ExitStack() as ctx:
        idx_tile = ctx.enter_context(
            nc.sbuf_tensor("idx_tile", [P, IDX_COLS], mybir.dt.int16)
        )
        g = ctx.enter_context(nc.sbuf_tensor("g", [P, TPP, DIM], mybir.dt.float32))
        dummy_idx = ctx.enter_context(
            nc.sbuf_tensor("dummy_idx", [P, 8], mybir.dt.int16)
        )
        scratch = ctx.enter_context(
            nc.sbuf_tensor("scratch", [P, 1, DIM], mybir.dt.float32)
        )
        s_idx = ctx.enter_context(nc.semaphore("s_idx"))
        s_warm = ctx.enter_context(nc.semaphore("s_warm"))
        s_ms = ctx.enter_context(nc.semaphore("s_ms"))
        s_gs = [ctx.enter_context(nc.semaphore(f"s_g{c}")) for c in range(chunks)]
        s_out = ctx.enter_context(nc.semaphore("s_out"))

        n_regs = {}
        for s in dict.fromkeys(sizes):
            n_regs[s] = nc.gpsimd.to_reg(s)

        nc.gpsimd.memset(dummy_idx[:], 0).then_inc(s_ms, 1)
        nc.gpsimd.wait_ge(s_ms, 1)
        nc.gpsimd.dma_gather(
            scratch[:], w[:], dummy_idx[:], P, P, DIM, queue_num=0
        ).then_inc(s_warm, 16)

        nc.sync.dma_start(idx_tile[:], x[:]).then_inc(s_idx, 16)

        for k in range(loop_m):
            if no_idx:
                if k > 0:
                    # serialize iterations + WAR-protect g without an idx DMA
                    nc.gpsimd.wait_ge(
                        s_out if not no_wb else s_gs[-1],
                        (16 * chunks * k) if not no_wb else 16 * k,
                    )
                nc.gpsimd.wait_ge(s_idx, 16)
            else:
                if k > 0:
                    if no_wb:
                        for c in range(chunks):
                            nc.sync.wait_ge(s_gs[c], 16 * k)
                    else:
                        nc.sync.wait_ge(s_out, 16 * chunks * k)
                    nc.sync.dma_start(idx_tile[:], x[:]).then_inc(s_idx, 16)
                nc.gpsimd.wait_ge(s_idx, 16 * (k + 1))
            for c in range(chunks):
                j0, j1 = bounds[c] // P, bounds[c + 1] // P
                nc.gpsimd.dma_gather(
                    g[:, j0:j1, :],
                    w[:],
                    idx_tile[:, bounds[c] // 16 : bounds[c + 1] // 16],
                    sizes[c],
                    n_regs[sizes[c]],
                    DIM,
                    queue_num=c % n_queues,
                ).then_inc(s_gs[c], 16)
            if not no_wb:
                for c in range(chunks):
                    j0, j1 = bounds[c] // P, bounds[c + 1] // P
                    nc.sync.wait_ge(s_gs[c], 16 * (k + 1))
                    nc.sync.dma_start(
                        out[:, j0:j1, :], g[:, j0:j1, :]
                    ).then_inc(s_out, 16)
        if no_wb:
            # write the output once at the end so the ExternalOutput has a
            # writer (walrus requires semaphore updates per DMA anyway)
            for c in range(chunks):
                nc.sync.wait_ge(s_gs[c], 16 * loop_m)
            nc.sync.dma_start(out[:], g[:]).then_inc(s_out, 16)
    nc.compile()
    return nc


def build_nc_v4mq(loop_m: int = 1):
    # merged writeback for the two 128-token primer chunks
    return build_nc_v4(loop_m, sizes=(128, 128, 256, 512), n_queues=4,
                       wb_groups=[(0, 1), (2,), (3,)])


def build_nc_v4sp(loop_m: int = 1):
    return build_nc_v4(loop_m, sizes=(128, 128, 256, 512), n_queues=4,
                       single_packet=False)


def build_nc_v4hq(loop_m: int = 1):
    # tail-light ordering: big chunk third, tiny chunk last so the final
    # drain + writeback are short
    return build_nc_v4(loop_m, sizes=(128, 256, 512, 128), n_queues=4)


def build_nc_v4eq_noidx(loop_m: int = 1):
    return _probe_builder(loop_m, no_idx=True)


def build_nc_v4eq_nowb(loop_m: int = 1):
    return _probe_builder(loop_m, no_wb=True)


def build_nc_v4eq_gonly(loop_m: int = 1):
    return _probe_builder(loop_m, no_idx=True, no_wb=True)


def build_nc_v4e(loop_m: int = 1):
    return build_nc_v4(loop_m, sizes=(128, 128, 256, 512))


def build_nc_v4cq(loop_m: int = 1):
    return build_nc_v4(loop_m, sizes=(512, 256, 256), n_queues=3)


def build_nc_v4q2(loop_m: int = 1):
    return build_nc_v4(loop_m, sizes=(512, 512), n_queues=2)


def build_nc_v4dq(loop_m: int = 1):
    return build_nc_v4(loop_m, sizes=(256, 256, 256, 256), n_queues=4)


def build_nc_v5(loop_m: int = 1, sizes=(TPP,), wb_groups=None):
    """v5: ONE batched vector-indirect DMA (InstDMACopy on qPoolDynamic)
    instead of gpsimd dma_gather ucode.

    dma_gather needs the `mlp` gpsimd library; its LOAD_LIB DMA (~8.8us) +
    warmup gather (~1.6us) sit on the one-shot critical path (the graded
    metric is a single NEFF execution, so the loop-harness amortization the
    v4 line was tuned for never happens). indirect_dma_start lowers to a
    plain DMA with DynamicAccessPatternInfo - descriptor expansion happens
    in the SWDGE base path, no library load, no ucode warmup.

    Index layout: idx_tile[p, j] = token p*TPP + j (int32, [128, 8]); each
    index gathers one contiguous 512B row of w; row i (offset-AP C-order
    p*TPP+j) lands at output block i = g[p, j, :]. Host reshape(TPC, DIM)
    restores token order. `sizes` splits the gather into chunks of TPP
    columns each so chunk c's HWDGE writeback overlaps chunk c+1's drain.
    """
    assert sum(sizes) == TPP
    orig_barrier = bass.Bass.all_engine_barrier
    bass.Bass.all_engine_barrier = lambda self, *a, **k: None
    try:
        nc = bacc.Bacc(
            None, target_bir_lowering=False, dynamic_dma_scratch_size=32768
        )
    finally:
        bass.Bass.all_engine_barrier = orig_barrier

    x = nc.dram_tensor("x", [P, TPP], mybir.dt.int32, kind="ExternalInput")
    w = nc.dram_tensor("weight", [VOCAB, DIM], mybir.dt.float32, kind="ExternalInput")
    out = nc.dram_tensor("out", [P, TPP, DIM], mybir.dt.float32, kind="ExternalOutput")

    chunks = len(sizes)
    bounds = [0]
    for s in sizes:
        bounds.append(bounds[-1] + s)

    with contextlib.ExitStack() as ctx:
        idx_tile = ctx.enter_context(
            nc.sbuf_tensor("idx_tile", [P, TPP], mybir.dt.int32)
        )
        g = ctx.enter_context(nc.sbuf_tensor("g", [P, TPP, DIM], mybir.dt.float32))
        s_idx = ctx.enter_context(nc.semaphore("s_idx"))
        s_gs = [ctx.enter_context(nc.semaphore(f"s_g{c}")) for c in range(chunks)]
        s_out = ctx.enter_context(nc.semaphore("s_out"))

        groups = wb_groups or [(c,) for c in range(chunks)]
        n_wbs = len(groups)
        for k in range(loop_m):
            if k > 0:
                nc.sync.wait_ge(s_out, 16 * n_wbs * k)
            nc.sync.dma_start(idx_tile[:], x[:]).then_inc(s_idx, 16)
            nc.gpsimd.wait_ge(s_idx, 16 * (k + 1))
            for c in range(chunks):
                j0, j1 = bounds[c], bounds[c + 1]
                nc.gpsimd.indirect_dma_start(
                    out=g[:, j0:j1, :],
                    out_offset=None,
                    in_=w[:],
                    in_offset=bass.IndirectOffsetOnAxis(
                        ap=idx_tile[:, j0:j1], axis=0
                    ),
                ).then_inc(s_gs[c], 16)
            for grp in groups:
                j0, j1 = bounds[grp[0]], bounds[grp[-1] + 1]
                for c in grp:
                    nc.sync.wait_ge(s_gs[c], 16 * (k + 1))
                nc.sync.dma_start(
                    out[:, j0:j1, :], g[:, j0:j1, :]
                ).then_inc(s_out, 16)
    nc.compile()
    return nc


IDXW = 64  # idx tile words/partition; 256B pitch is required for the
           # batched vector-indirect op to fetch one index per dest block
           # (32B pitch degrades to one index per partition + contiguous run)


def build_nc_v7(loop_m: int = 1, wb_groups=((0, TPP),)):
    """v7: ONE batched vector-indirect DMA for all 1024 rows.

    Probed on HW (dbg_walk.py): with the idx tile at 256B partition pitch
    ([P, 64] int32, first TPP words used) and a flat 2D dest [P, TPP*DIM],
    the SWDGE indirect1d expansion consumes exactly one index per 512B dest
    block in (p, j) C-order: g[p, j*DIM:(j+1)*DIM] = w[idx[p, j]]. One Pool
    dispatch (~1.1us) replaces 8 (~11.3us serial).

    wb_groups: list of (j0, j1) column ranges, one HWDGE writeback each,
    gather split per group so wb overlaps the later groups' drains.
    """
    orig_barrier = bass.Bass.all_engine_barrier
    bass.Bass.all_engine_barrier = lambda self, *a, **k: None
    try:
        nc = bacc.Bacc(
            None, target_bir_lowering=False, dynamic_dma_scratch_size=32768
        )
    finally:
        bass.Bass.all_engine_barrier = orig_barrier

    x = nc.dram_tensor("x", [P, IDXW], mybir.dt.int32, kind="ExternalInput")
    w = nc.dram_tensor("weight", [VOCAB, DIM], mybir.dt.float32, kind="ExternalInput")
    out = nc.dram_tensor("out", [P, TPP, DIM], mybir.dt.float32, kind="ExternalOutput")

    with contextlib.ExitStack() as ctx:
        idx_t = ctx.enter_context(nc.sbuf_tensor("idx", [P, IDXW], mybir.dt.int32))
        g_t = ctx.enter_context(
            nc.sbuf_tensor("g", [P, TPP * DIM], mybir.dt.float32)
        )
        s_idx = ctx.enter_context(nc.semaphore("s_idx"))
        s_gs = [ctx.enter_context(nc.semaphore(f"s_g{i}")) for i in range(len(wb_groups))]
        s_out = ctx.enter_context(nc.semaphore("s_out"))

        for k in range(loop_m):
            if k > 0:
                nc.sync.wait_ge(s_out, 16 * len(wb_groups) * k)
            nc.sync.dma_start(idx_t[:], x[:]).then_inc(s_idx, 16)
            nc.gpsimd.wait_ge(s_idx, 16 * (k + 1))
            for gi, (j0, j1) in enumerate(wb_groups):
                nc.gpsimd.indirect_dma_start(
                    out=g_t[:, j0 * DIM : j1 * DIM],
                    out_offset=None,
                    in_=w[:],
                    in_offset=bass.IndirectOffsetOnAxis(
                        ap=idx_t[:, j0:j1], axis=0
                    ),
                ).then_inc(s_gs[gi], 16)
            for gi, (j0, j1) in enumerate(wb_groups):
                nc.sync.wait_ge(s_gs[gi], 16 * (k + 1))
                nc.sync.dma_start(
                    out[:, j0:j1, :], g_t[:, j0 * DIM : j1 * DIM]
                ).then_inc(s_out, 16)
    nc.compile()
    return nc


def build_nc_v6(loop_m: int = 1, n_queues: int = 1,
                wb_groups=((0, 1, 2, 3), (4, 5, 6, 7)),
                col_order=None, shared_idx: bool = False,
                shared_g: bool = False, batched: bool = False,
                wb_engines=("sync",)):
    """v6: 8 per-column vector-indirect DMAs (the HW-proven expansion shape:
    ONE index per partition per op - idx[:, j:j+1] -> g[:, j, :]).

    No gpsimd ucode library, no LOAD_LIB (~8.8us) and no warmup (~1.6us) on
    the one-shot critical path. `n_queues` > 1 round-robins the ops across
    qPoolDynamic{i} SWDGE queues (desc-gen parallelizes across queues; the
    InstDMACopy queue attr is patched post-construction). wb_groups: column
    groups per HWDGE writeback, fired as soon as member gathers complete.
    """
    orig_barrier = bass.Bass.all_engine_barrier
    bass.Bass.all_engine_barrier = lambda self, *a, **k: None
    try:
        nc = bacc.Bacc(
            None, target_bir_lowering=False, dynamic_dma_scratch_size=32768,
            num_swdge_queues=n_queues,
        )
    finally:
        bass.Bass.all_engine_barrier = orig_barrier

    # shared_idx: x[p, j] = token p*TPP + j (one contiguous [P, TPP] load).
    # else: x[j, p] = token p*TPP + j so each column-op's [P, 1] idx load is
    # a contiguous 512B DRAM read.
    x_shape = [P, TPP] if shared_idx else [TPP, P]
    x = nc.dram_tensor("x", x_shape, mybir.dt.int32, kind="ExternalInput")
    w = nc.dram_tensor("weight", [VOCAB, DIM], mybir.dt.float32, kind="ExternalInput")
    out = nc.dram_tensor("out", [P, TPP, DIM], mybir.dt.float32, kind="ExternalOutput")

    cols = col_order or list(range(TPP))

    with contextlib.ExitStack() as ctx:
        # Baseline HW-proven shape (tile_scatter_add): per-op dest tile
        # [P, DIM] 2D at offset 0, per-op idx tile [P, 1] 2D at offset 0.
        # shared_idx / shared_g probe whether sliced (offset != 0) 2D APs
        # also work, which collapses the 8 idx loads / 8 writebacks.
        if shared_idx:
            idx_t = ctx.enter_context(
                nc.sbuf_tensor("idx", [P, TPP], mybir.dt.int32)
            )
            idx_aps = [idx_t[:, j : j + 1] for j in range(TPP)]
        else:
            idx_tiles = [
                ctx.enter_context(
                    nc.sbuf_tensor(f"idx{j}", [P, 1], mybir.dt.int32)
                )
                for j in range(TPP)
            ]
            idx_aps = [t[:] for t in idx_tiles]
        if shared_g:
            g_t = ctx.enter_context(
                nc.sbuf_tensor("g", [P, TPP * DIM], mybir.dt.float32)
            )
            g_aps = [g_t[:, j * DIM : (j + 1) * DIM] for j in range(TPP)]
        else:
            g_tiles = [
                ctx.enter_context(
                    nc.sbuf_tensor(f"g{j}", [P, DIM], mybir.dt.float32)
                )
                for j in range(TPP)
            ]
            g_aps = [t[:] for t in g_tiles]
        s_idx = ctx.enter_context(nc.semaphore("s_idx"))
        s_gs = [ctx.enter_context(nc.semaphore(f"s_g{j}")) for j in range(TPP)]
        s_out = ctx.enter_context(nc.semaphore("s_out"))

        n_wbs = len(wb_groups)
        for k in range(loop_m):
            if k > 0:
                nc.sync.wait_ge(s_out, 16 * n_wbs * k)
            if shared_idx:
                nc.sync.dma_start(idx_t[:], x[:]).then_inc(s_idx, 16)
                idx_target = 16
            else:
                for j in range(TPP):
                    nc.sync.dma_start(
                        idx_aps[j], x[j : j + 1, :].rearrange("a b -> b a")
                    ).then_inc(s_idx, 16)
                idx_target = 16 * TPP
            nc.gpsimd.wait_ge(s_idx, idx_target * (k + 1))
            if batched:
                # ONE 2D-everything op: dest [P, TPP*DIM] iterates TPP
                # 512B blocks per partition in lockstep with idx [P, TPP]
                assert shared_idx and shared_g
                for gi, grp in enumerate(wb_groups):
                    j0, j1 = grp[0], grp[-1] + 1
                    inst = nc.gpsimd.indirect_dma_start(
                        out=g_t[:, j0 * DIM : j1 * DIM],
                        out_offset=None,
                        in_=w[:],
                        in_offset=bass.IndirectOffsetOnAxis(
                            ap=idx_t[:, j0:j1], axis=0
                        ),
                    )
                    q = gi % n_queues
                    if q:
                        inst.ins.queue = f"qPoolDynamic{q}"
                    inst.then_inc(s_gs[grp[0]], 16)
            else:
                for i, j in enumerate(cols):
                    inst = nc.gpsimd.indirect_dma_start(
                        out=g_aps[j],
                        out_offset=None,
                        in_=w[:],
                        in_offset=bass.IndirectOffsetOnAxis(
                            ap=idx_aps[j], axis=0
                        ),
                    )
                    q = i % n_queues
                    if q:
                        inst.ins.queue = f"qPoolDynamic{q}"
                    inst.then_inc(s_gs[j], 16)
            engs = {"sync": nc.sync, "act": nc.scalar}
            for gi, grp in enumerate(wb_groups):
                eng = engs[wb_engines[gi % len(wb_engines)]]
                for j in (grp[:1] if batched else grp):
                    eng.wait_ge(s_gs[j], 16 * (k + 1))
                if shared_g:
                    j0, j1 = grp[0], grp[-1] + 1
                    eng.dma_start(
                        out[:, j0:j1, :], g_t[:, j0 * DIM : j1 * DIM]
                    ).then_inc(s_out, 16)
                else:
                    for j in grp:
                        eng.dma_start(
                            out[:, j : j + 1, :], g_aps[j]
                        ).then_inc(s_out, 16)
    nc.compile()
    return nc


def build_nc_v8(loop_m: int = 1, sizes=(128, 128, 256, 512), n_queues=4,
                wb_groups=((0, 1), (2,), (3,)), warmup=False,
                seq_codegen=False):
    """v8: one-shot-optimized dma_gather. The mlp ucode library load
    (~8.8us) is dispatched as the FIRST Pool instruction (explicit
    load_library) so it fully overlaps the idx DMA + its latency; no
    warmup gather (its ~1.6us serial cost is not worth it one-shot - the
    first real gather pays the cold cost while later chunks' desc-gen
    queues behind it anyway). Chunked gathers on separate SWDGE queues
    (v4 lesson: dispatch is ~70ns when a Q7 core is free; chunks must be
    pow2 sizes; ascending order primes writeback earliest)."""
    assert sum(sizes) == TPC and all(s % 128 == 0 for s in sizes)
    orig_barrier = bass.Bass.all_engine_barrier
    bass.Bass.all_engine_barrier = lambda self, *a, **k: None
    try:
        nc = bacc.Bacc(
            None, target_bir_lowering=False, dynamic_dma_scratch_size=32768,
            num_swdge_queues=n_queues, use_seq_codegen=seq_codegen,
        )
    finally:
        bass.Bass.all_engine_barrier = orig_barrier

    x = nc.dram_tensor("x", [P, IDX_COLS], mybir.dt.int16, kind="ExternalInput")
    w = nc.dram_tensor("weight", [VOCAB, DIM], mybir.dt.float32, kind="ExternalInput")
    out = nc.dram_tensor("out", [P, TPP, DIM], mybir.dt.float32, kind="ExternalOutput")

    chunks = len(sizes)
    bounds = [0]
    for s in sizes:
        bounds.append(bounds[-1] + s)

    with contextlib.ExitStack() as ctx:
        idx_tile = ctx.enter_context(
            nc.sbuf_tensor("idx_tile", [P, IDX_COLS], mybir.dt.int16)
        )
        g = ctx.enter_context(nc.sbuf_tensor("g", [P, TPP, DIM], mybir.dt.float32))
        dummy_idx = ctx.enter_context(
            nc.sbuf_tensor("dummy_idx", [P, 8], mybir.dt.int16)
        )
        scratch = ctx.enter_context(
            nc.sbuf_tensor("scratch", [P, 1, DIM], mybir.dt.float32)
        )
        s_idx = ctx.enter_context(nc.semaphore("s_idx"))
        s_ms = ctx.enter_context(nc.semaphore("s_ms"))
        s_warm = ctx.enter_context(nc.semaphore("s_warm"))
        s_gs = [ctx.enter_context(nc.semaphore(f"s_g{c}")) for c in range(chunks)]
        s_out = ctx.enter_context(nc.semaphore("s_out"))

        # start the ucode library DMA immediately; it runs while the idx
        # DMA + HBM latency elapse
        nc.gpsimd.load_library(library_config.mlp)

        n_regs = {}
        for s in dict.fromkeys(sizes):
            n_regs[s] = nc.gpsimd.to_reg(s)

        if warmup:
            nc.gpsimd.memset(dummy_idx[:], 0).then_inc(s_ms, 1)
            nc.gpsimd.wait_ge(s_ms, 1)
            nc.gpsimd.dma_gather(
                scratch[:], w[:], dummy_idx[:], P, P, DIM, queue_num=0
            ).then_inc(s_warm, 16)

        nc.sync.dma_start(idx_tile[:], x[:]).then_inc(s_idx, 16)

        n_wbs = len(wb_groups)
        for k in range(loop_m):
            if k > 0:
                nc.sync.wait_ge(s_out, 16 * n_wbs * k)
                nc.sync.dma_start(idx_tile[:], x[:]).then_inc(s_idx, 16)
            nc.gpsimd.wait_ge(s_idx, 16 * (k + 1))
            for c in range(chunks):
                j0, j1 = bounds[c] // P, bounds[c + 1] // P
                nc.gpsimd.dma_gather(
                    g[:, j0:j1, :],
                    w[:],
                    idx_tile[:, bounds[c] // 16 : bounds[c + 1] // 16],
                    sizes[c],
                    n_regs[sizes[c]],
                    DIM,
                    queue_num=c % n_queues,
                ).then_inc(s_gs[c], 16)
            for grp in wb_groups:
                j0 = bounds[grp[0]] // P
                j1 = bounds[grp[-1] + 1] // P
                for c in grp:
                    nc.sync.wait_ge(s_gs[c], 16 * (k + 1))
                nc.sync.dma_start(
                    out[:, j0:j1, :], g[:, j0:j1, :]
                ).then_inc(s_out, 16)
    nc.compile()
    return nc


VSH = VOCAB // N_CORES   # 4000 vocab rows per core (vocab-sharded table)
NTOK = 1536              # padded per-core token capacity (exp ~1024, 8.5+sigma)


def build_nc_v9(loop_m: int = 1, n_chunks: int = 1):
    """v9: vocab-sharded ap_gather (SBUF-resident transposed table shard).

    Core c owns vocab rows [c*VSH, (c+1)*VSH); host routes each token to its
    owning core (the sharding_hint's vocab-parallel scheme) and un-permutes
    on the way out. Pipeline per core:
      1. load_library(ap_gather) - small lib, ~2.3us clean
      2. tiny warmup ap_gather; its completion sem doubles as the only
         observable "library loaded" signal, gating the big shard DMA (a
         2MB DMA concurrent with the Q7 library load starves the loader:
         measured 43us vs 2.3us)
      3. WT shard [128 dim, VSH] fp32 DMA -> SBUF (~6.5us at 313GB/s)
      4. one ap_gather: g[dim_p, k] = WT[dim_p, loc_idx[k]], all NTOK tokens
      5. HWDGE writeback g [128, NTOK] -> out DRAM (host transposes back)
    n_chunks > 1 splits 3-5 into vocab-range chunks (host buckets tokens
    per chunk) so shard-DMA / gather / writeback pipeline.
    """
    orig_barrier = bass.Bass.all_engine_barrier
    bass.Bass.all_engine_barrier = lambda self, *a, **k: None
    try:
        nc = bacc.Bacc(
            None, target_bir_lowering=False, dynamic_dma_scratch_size=32768
        )
    finally:
        bass.Bass.all_engine_barrier = orig_barrier

    assert VSH % n_chunks == 0 and NTOK % n_chunks == 0
    vch = VSH // n_chunks
    tch = NTOK // n_chunks

    wt = nc.dram_tensor("wt", [P, VSH], mybir.dt.float32, kind="ExternalInput")
    xw = nc.dram_tensor("xw", [P, NTOK // 16], mybir.dt.int16, kind="ExternalInput")
    out = nc.dram_tensor("out", [P, NTOK], mybir.dt.float32, kind="ExternalOutput")

    with contextlib.ExitStack() as ctx:
        wt_t = ctx.enter_context(nc.sbuf_tensor("wt_t", [P, VSH, 1], mybir.dt.float32))
        idx_t = ctx.enter_context(
            nc.sbuf_tensor("idx_t", [P, NTOK // 16], mybir.dt.int16)
        )
        g_t = ctx.enter_context(nc.sbuf_tensor("g_t", [P, NTOK, 1], mybir.dt.float32))
        dummy = ctx.enter_context(nc.sbuf_tensor("dmy_idx", [P, 1], mybir.dt.int16))
        scr = ctx.enter_context(nc.sbuf_tensor("scr", [P, 16, 1], mybir.dt.float32))
        s_ms = ctx.enter_context(nc.semaphore("s_ms"))
        s_lib = ctx.enter_context(nc.semaphore("s_lib"))
        s_idx = ctx.enter_context(nc.semaphore("s_idx"))
        s_wts = [ctx.enter_context(nc.semaphore(f"s_wt{c}")) for c in range(n_chunks)]
        s_gs = [ctx.enter_context(nc.semaphore(f"s_g{c}")) for c in range(n_chunks)]
        s_out = ctx.enter_context(nc.semaphore("s_out"))

        nc.gpsimd.load_library(library_config.ap_gather)
        # lib-loaded gate: first custom op blocks until the library lands
        nc.gpsimd.memset(dummy[:], 0).then_inc(s_ms, 1)
        nc.gpsimd.wait_ge(s_ms, 1)
        nc.gpsimd.ap_gather(
            scr[:], wt_t[:, :16, :], dummy[:], channels=P, num_elems=16, d=1,
            num_idxs=16,
        ).then_inc(s_lib, 1)

        # idx DMA is tiny - safe concurrent with the library load
        nc.sync.dma_start(idx_t[:], xw[:]).then_inc(s_idx, 16)
        nc.sync.wait_ge(s_lib, 1)

        for k in range(loop_m):
            if k > 0:
                nc.sync.wait_ge(s_out, 16 * n_chunks * k)
            for c in range(n_chunks):
                nc.sync.dma_start(
                    wt_t[:, c * vch : (c + 1) * vch, 0],
                    wt[:, c * vch : (c + 1) * vch],
                ).then_inc(s_wts[c], 16)
            nc.gpsimd.wait_ge(s_idx, 16)
            for c in range(n_chunks):
                nc.gpsimd.wait_ge(s_wts[c], 16 * (k + 1))
                nc.gpsimd.ap_gather(
                    g_t[:, c * tch : (c + 1) * tch, :],
                    wt_t[:, c * vch : (c + 1) * vch, :],
                    idx_t[:, c * (tch // 16) : (c + 1) * (tch // 16)],
                    channels=P,
                    num_elems=vch,
                    d=1,
                    num_idxs=tch,
                ).then_inc(s_gs[c], 1)
            for c in range(n_chunks):
                nc.sync.wait_ge(s_gs[c], k + 1)
                nc.sync.dma_start(
                    out[:, c * tch : (c + 1) * tch],
                    g_t[:, c * tch : (c + 1) * tch, 0],
                ).then_inc(s_out, 16)
    nc.compile()
    return nc


def _v9_in_maps(x_flat: np.ndarray, w: np.ndarray):
    """Route tokens to vocab-owning cores; return in_maps + unpermute info."""
    owner = x_flat // VSH                       # owning core per token
    order = np.argsort(owner, kind="stable")    # token positions grouped by core
    counts = np.bincount(owner, minlength=N_CORES)
    assert counts.max() <= NTOK, f"token bucket overflow: {counts.max()} > {NTOK}"
    in_maps = []
    for c in range(N_CORES):
        sel = order[counts[:c].sum() : counts[: c + 1].sum()]
        loc = (x_flat[sel] - c * VSH).astype(np.int16)
        locp = np.zeros(NTOK, dtype=np.int16)
        locp[: len(sel)] = loc
        t16 = locp.reshape(NTOK // 16, 16).T    # wrap for gpsimd stripes
        in_maps.append({
            "wt": np.ascontiguousarray(w[c * VSH : (c + 1) * VSH, :].T),
            "xw": np.ascontiguousarray(np.tile(t16, (P // 16, 1))),
        })
    return in_maps, order, counts


_NC_CACHE = None


def _wrap_idxs(tokens: np.ndarray) -> np.ndarray:
    """dma_gather idx layout: idx[p, s] = tokens[s*16 + p%16], [128, 64] i16."""
    t16 = tokens.reshape(IDX_COLS, 16).T.astype(np.int16)  # [16, 64]
    return np.ascontiguousarray(np.tile(t16, (P // 16, 1)))


_IOTA_WRAPPED = None


def _wrap_iota() -> np.ndarray:
    global _IOTA_WRAPPED
    if _IOTA_WRAPPED is None:
        _IOTA_WRAPPED = _wrap_idxs(np.arange(TPC, dtype=np.int64))
    return _IOTA_WRAPPED


def bench_in_maps(inputs):
    x_flat = inputs["x"].reshape(-1).astype(np.int64)
    w = np.ascontiguousarray(inputs["weight"].astype(np.float32))
    return [
        {
            "x": _wrap_idxs(x_flat[c * TPC : (c + 1) * TPC]),
            "weight": w,
            "wb_idx": _wrap_iota(),
        }
        for c in range(N_CORES)
    ]


def kernel(x: np.ndarray, weight: np.ndarray, **run_kwargs):
    global _NC_CACHE
    if _NC_CACHE is None:
        _NC_CACHE = build_nc_v6(
            shared_idx=True, shared_g=True, n_queues=1,
            wb_groups=((0, 1, 2, 3, 4, 5), (6, 7)),
            wb_engines=("sync", "act"),
        )
    nc = _NC_CACHE

    x_flat = np.asarray(x).reshape(-1).astype(np.int64)
    w = np.ascontiguousarray(np.asarray(weight, dtype=np.float32))

    in_maps = [
        {
            # shared_idx layout: x[p, j] = token p*TPP + j (see build_nc_v6)
            "x": np.ascontiguousarray(
                x_flat[c * TPC : (c + 1) * TPC].reshape(P, TPP).astype(np.int32)
            ),
            "weight": w,
        }
        for c in range(N_CORES)
    ]
    res = run_bass_kernel_spmd(nc, in_maps, core_ids=list(range(N_CORES)), **run_kwargs)
    # out [128, 8, 128]: token p*8+j lives at [p, j, :] -> plain reshape
    parts = [res.results[c]["out"].reshape(TPC, DIM) for c in range(N_CORES)]
    full = np.concatenate(parts, axis=0).reshape(B, S, DIM)
    if run_kwargs:
        return full, res
    return full
